# revision 1
# baseline (speedup 1.0000x reference)
"""GQA attention kernel for 8 Trainium2 cores (v2).

Problem: B=2, T=2048, D=2048, 32 q-heads, 8 kv-heads, head_dim=64, causal.

Sharding: core c = (b, jg) with b = c//4, jg = c%4. Each core handles batch b,
kv-heads {2jg, 2jg+1} and q-heads {8jg..8jg+7} (data parallel on B, tensor
parallel on heads; wq/wk/wv column-sharded, wo row-sharded). Each core returns
a partial output projection resT [D, T]; the host sums the 4 partials per
batch and transposes.

v2 design changes vs the 406us baseline:
 - input DMAs spread over the SP/ACT/POOL queues with wkv first so the first
   projection matmul starts at ~14us instead of ~44us.
 - k/q [t,f]->[f,t] transposes moved off the PE onto the DMA XBAR
   (dma_start_transpose), eliminating PE transpose+copy traffic.
 - AV matmul transposed: out[t(128), f(65)] accumulates with P-tiles as the
   stationary operand and [V|1] as the 65-row moving operand (65 rows vs 512
   rows per s-tile on the PE). The 65th column of the accumulator is the
   softmax denominator, so no separate reduction and no DRAM round-trip
   broadcast: reciprocal via ACT Ln/Exp on a [128,4,1] column, applied as a
   free-dim step-0 broadcast multiply.
 - PSUM accumulators for the 4 t-blocks share one bank; groups cannot use
   start=True (2KB zero-region granularity), so the bank is DVE-memset to 0
   and all AV matmuls accumulate with start=False.
 - causal diagonal computed at 128-column granularity (only the lower
   trapezoid), saving ~37% of diagonal scores/exp/AV work; only the true
   diagonal 128x128 subtiles get the multiplicative tril mask (on gpsimd).
 - output-projection results staged through SBUF (DMA cannot read PSUM) and
   streamed out per 128-row tile on the idle SP/POOL queues.
"""

import os
import sys

sys.path.insert(0, "/opt/trn_rl_repo")

import json

import numpy as np
import ml_dtypes

import concourse.bass as bass
import concourse.mybir as mybir
from concourse.tile import TileContext
from concourse.masks import make_identity
from concourse.bass_utils import run_bass_kernel_spmd

BF16 = mybir.dt.bfloat16
FP8 = mybir.dt.float8e4
F32 = mybir.dt.float32

T = 2048
D = 2048
HD = 64
NCORES = 8
KT = D // 128           # 16 contraction tiles
NTT = T // 128          # 16 time tiles
NCH = T // 512          # 4 chunks
NBF = ml_dtypes.bfloat16

# ---------------------------------------------------------------------------
# BIR post-pass: split multi-wait instructions into single-wait
# EventSemaphore carriers (the walrus build here allows one wait per inst).
# ---------------------------------------------------------------------------
_ws_ctr = [0]


def _split_waits_bytes(bir: bytes) -> bytes:
    d = json.loads(bir)
    for f in d.get("functions", []):
        for bb in f.get("blocks", []):
            out = []
            for inst in bb.get("instructions", []):
                si = inst.get("sync_info")
                waits = (si or {}).get("on_wait") or []
                if len(waits) > 1:
                    for w in waits[:-1]:
                        _ws_ctr[0] += 1
                        out.append({
                            "debug": inst.get("debug", 0),
                            "engine": inst["engine"],
                            "ins": [],
                            "name": f"WS-{_ws_ctr[0]}",
                            "opcode": "EventSemaphore",
                            "outs": [],
                            "sync_info": {"on_update": [], "on_wait": [w]},
                        })
                    si["on_wait"] = [waits[-1]]
                out.append(inst)
            bb["instructions"] = out
    return json.dumps(d).encode()


def _install_waitsplit():
    import concourse.bass2jax as b2j

    if getattr(b2j, "_waitsplit_installed", False):
        return
    orig = b2j._decompress_ant_bir
    b2j._decompress_ant_bir = lambda s: _split_waits_bytes(orig(s))
    b2j._waitsplit_installed = True


# ---------------------------------------------------------------------------
# Device program
# ---------------------------------------------------------------------------

def _bcast(ap2d, nh):
    """Insert a step-0 head dim into a [p, w] AP -> [p, nh, w]."""
    return bass.AP(tensor=ap2d.tensor, offset=ap2d.offset,
                   ap=[ap2d.ap[0], [0, nh], ap2d.ap[1]])


def _bcast_last(ap3d, w):
    """Append a step-0 last dim to a [p, n, 1] AP -> [p, n, w]."""
    return bass.AP(tensor=ap3d.tensor, offset=ap3d.offset,
                   ap=[ap3d.ap[0], ap3d.ap[1], [0, w]])


def _build(causal: bool):
    nc = bass.Bass()
    xt = nc.dram_tensor("xt", [D, T], BF16, kind="ExternalInput")
    wq = nc.dram_tensor("wq", [D, 512], BF16, kind="ExternalInput")
    wkv = nc.dram_tensor("wkv", [D, 256], BF16, kind="ExternalInput")
    wo = nc.dram_tensor("wo", [512, D], BF16, kind="ExternalInput")
    cexp = nc.dram_tensor("cexp", [T, 64], F32, kind="ExternalInput")
    sexp = nc.dram_tensor("sexp", [T, 64], F32, kind="ExternalInput")
    res = nc.dram_tensor("res", [D, T], F32, kind="ExternalOutput")

    with TileContext(nc) as tc:
        with (
            tc.tile_pool(name="const", bufs=1) as const,
            tc.tile_pool(name="big", bufs=1) as big,
            tc.tile_pool(name="ropew", bufs=3) as ropew,
            tc.tile_pool(name="qtfp", bufs=3) as qtfp,
            tc.tile_pool(name="ptp", bufs=4) as ptp,
            tc.tile_pool(name="ohp", bufs=3) as ohp,
            tc.tile_pool(name="rp", bufs=3) as rp,
            tc.tile_pool(name="ohtp", bufs=6) as ohtp,
            tc.tile_pool(name="rsp", bufs=3) as rsp,
            tc.tile_pool(name="pmm", bufs=2, space="PSUM") as pmm,
            tc.tile_pool(name="psc", bufs=2, space="PSUM") as psc,
            tc.tile_pool(name="pav", bufs=2, space="PSUM") as pavp,
        ):
            # ---------------- constants ----------------
            ident = const.tile([128, 128], BF16)
            make_identity(nc, ident)

            mtri = None
            if causal:
                # keep col >= row (upper triangle incl diagonal) of a
                # [s_local, t_local] 128x128 tile
                mtri = const.tile([128, 128], BF16)
                nc.vector.memset(mtri, 1.0)
                nc.gpsimd.affine_select(
                    out=mtri, in_=mtri, pattern=[[1, 128]], base=0,
                    channel_multiplier=-1, compare_op=mybir.AluOpType.is_ge,
                    fill=0.0)

            # ---------------- weights + x loads, spread over 3 queues ------
            wkv_sb = big.tile([128, KT, 256], BF16)
            nc.sync.dma_start(out=wkv_sb, in_=wkv.rearrange("(n p) c -> p n c", p=128))

            cexp_sb = const.tile([128, NTT, 64], F32)
            sexp_sb = const.tile([128, NTT, 64], F32)
            nc.scalar.dma_start(out=cexp_sb, in_=cexp.rearrange("(n p) c -> p n c", p=128))
            nc.scalar.dma_start(out=sexp_sb, in_=sexp.rearrange("(n p) c -> p n c", p=128))

            wq_sb = big.tile([128, KT, 512], BF16)
            nc.gpsimd.dma_start(out=wq_sb, in_=wq.rearrange("(n p) c -> p n c", p=128))

            xt_sb = big.tile([128, KT, T], BF16)
            xq = [nc.sync, nc.scalar, nc.gpsimd]

            wo_sb = big.tile([128, 4, D], BF16)

            kT_sb = big.tile([128, NTT, 128], BF16)
            qT_all = big.tile([128, NTT, 512], BF16)
            # [kv0 | 1 | kv1 | 1]: v features for both kv heads plus the
            # all-ones denominator columns, one copy per s-tile
            vp = big.tile([128, NTT, 2, 65], BF16)
            nc.vector.memset(vp[:, :, :, 64:65], 1.0)

            def rope(ps3, out_bf, ti, nh):
                """RoPE in [t, f] layout. ps3: PSUM [128, nh, 64] f32 view ->
                out_bf: SBUF [128, nh*64] bf16. One DVE op to drain PSUM
                fast; the arithmetic runs on gpsimd from SBUF."""
                o3 = out_bf.rearrange("p (h k) -> p h k", h=nh)
                tmp = ropew.tile([128, nh, 64], F32, tag="ropeT", name="rt")
                nc.vector.tensor_copy(tmp, ps3)
                a = ropew.tile([128, nh, 64], F32, tag="ropeA", name="ra")
                bt = ropew.tile([128, nh, 64], F32, tag="ropeB", name="rb")
                cb = _bcast(cexp_sb[:, ti, :], nh)
                nc.gpsimd.tensor_tensor(out=a, in0=tmp, in1=cb, op=mybir.AluOpType.mult)
                sb_lo = _bcast(sexp_sb[:, ti, 0:32], nh)
                sb_hi = _bcast(sexp_sb[:, ti, 32:64], nh)
                nc.gpsimd.tensor_tensor(out=bt[:, :, 0:32], in0=tmp[:, :, 32:64],
                                        in1=sb_lo, op=mybir.AluOpType.mult)
                nc.gpsimd.tensor_tensor(out=bt[:, :, 32:64], in0=tmp[:, :, 0:32],
                                        in1=sb_hi, op=mybir.AluOpType.mult)
                nc.gpsimd.tensor_tensor(out=o3, in0=a, in1=bt, op=mybir.AluOpType.add)

            # ------- projections for one quarter of the time axis ----------
            def load_quarter(c):
                lo, hi = c * 512, (c + 1) * 512
                for kt in range(KT):
                    xq[kt % 3].dma_start(out=xt_sb[:, kt, lo:hi],
                                         in_=xt[kt * 128:(kt + 1) * 128, lo:hi])
                if c == 0:
                    # wo is first needed by chunk 0's output projection; load
                    # it behind the quarter-0 x slices, spread over all queues
                    for g in range(4):
                        xq[g % 3].dma_start(
                            out=wo_sb[:, g, :], in_=wo[g * 128:(g + 1) * 128, :])

            def proj_tile_kv(st):
                ps_kv = pmm.tile([128, 256], F32, tag="mm", name="pskv")
                for kt in range(KT):
                    nc.tensor.matmul(ps_kv, xt_sb[:, kt, st * 128:(st + 1) * 128],
                                     wkv_sb[:, kt, :], start=(kt == 0),
                                     stop=(kt == KT - 1))
                ktf = qtfp.tile([128, 128], BF16, tag="qtf", name="ktf")
                rope(ps_kv[:, 0:128].rearrange("p (h k) -> p h k", h=2),
                     ktf, st, 2)
                nc.sync.dma_start_transpose(kT_sb[:, st, :], ktf)
                nc.vector.tensor_copy(
                    vp[:, st, :, 0:64],
                    ps_kv[:, 128:256].rearrange("p (h k) -> p h k", h=2))

            def proj_tile_q(ti):
                ps_q = pmm.tile([128, 512], F32, tag="mm", name="psq")
                for kt in range(KT):
                    nc.tensor.matmul(ps_q, xt_sb[:, kt, ti * 128:(ti + 1) * 128],
                                     wq_sb[:, kt, :], start=(kt == 0),
                                     stop=(kt == KT - 1))
                qtf = qtfp.tile([128, 512], BF16, tag="qtf", name="qtf")
                rope(ps_q.rearrange("p (h k) -> p h k", h=8), qtf, ti, 8)
                c, tt = ti // 4, ti % 4
                for g in range(4):
                    nc.sync.dma_start_transpose(
                        qT_all[:, 4 * c + g, tt * 128:(tt + 1) * 128],
                        qtf[:, g * 128:(g + 1) * 128])


            # ------- attention + output projection for one 512-wide chunk --
            def attention_chunk(j, splice_q):
                ohT_all = ohtp.tile([128, 4, 512], BF16, tag="oht", name="oht")
                for g in range(4):
                    qT = qT_all[:, 4 * j + g, :]
                    poht = None
                    for half in (0, 1):
                        hb = 64 * half
                        pav = pavp.tile([128, 4, 128], F32, tag="av", name="pav")
                        nc.vector.memset(pav[:, :, 0:65], 0.0)

                        def av(ptile, si, tb0):
                            for tb in range(tb0, 4):
                                nc.tensor.matmul(
                                    pav[:, tb, 0:65],
                                    ptile[:, (tb - tb0) * 128:(tb - tb0 + 1) * 128],
                                    vp[:, si, half, :], start=False, stop=False,
                                    skip_group_check=True)

                        ns_off = 4 * j if causal else NTT
                        for sp in range(0, ns_off, 2):
                            ps2 = psc.tile([128, 2, 512], F32, tag="sc", name="pss")
                            for u in (0, 1):
                                nc.tensor.matmul(
                                    ps2[:, u, :], kT_sb[hb:hb + 64, sp + u, :],
                                    qT[hb:hb + 64, :],
                                    start=True, stop=True, skip_group_check=True)
                            pt2 = ptp.tile([128, 2, 512], BF16, tag="pt", name="pt")
                            nc.scalar.activation(out=pt2, in_=ps2,
                                                 func=mybir.ActivationFunctionType.Exp,
                                                 scale=0.125)
                            for u in (0, 1):
                                av(pt2[:, u, :], sp + u, 0)

                        if causal:
                            s0 = 4 * j
                            psd1 = psc.tile([128, 2, 512], F32, tag="sc", name="psd1")
                            psd2 = psc.tile([128, 2, 512], F32, tag="sc", name="psd2")
                            nc.tensor.matmul(psd1[:, 0, :],
                                             kT_sb[hb:hb + 64, s0, :],
                                             qT[hb:hb + 64, :],
                                             start=True, stop=True,
                                             skip_group_check=True)
                            nc.tensor.matmul(psd1[:, 1, 0:384],
                                             kT_sb[hb:hb + 64, s0 + 1, :],
                                             qT[hb:hb + 64, 128:512],
                                             start=True, stop=True,
                                             skip_group_check=True)
                            nc.tensor.matmul(psd1[:, 1, 384:512],
                                             kT_sb[hb:hb + 64, s0 + 3, :],
                                             qT[hb:hb + 64, 384:512],
                                             start=True, stop=True,
                                             skip_group_check=True)
                            nc.tensor.matmul(psd2[:, 0, 0:256],
                                             kT_sb[hb:hb + 64, s0 + 2, :],
                                             qT[hb:hb + 64, 256:512],
                                             start=True, stop=True,
                                             skip_group_check=True)
                            pd1 = ptp.tile([128, 2, 512], BF16, tag="pt", name="pd1")
                            nc.scalar.activation(out=pd1, in_=psd1,
                                                 func=mybir.ActivationFunctionType.Exp,
                                                 scale=0.125)
                            pd2 = ptp.tile([128, 2, 512], BF16, tag="pt", name="pd2")
                            nc.scalar.activation(out=pd2[:, 0, 0:256],
                                                 in_=psd2[:, 0, 0:256],
                                                 func=mybir.ActivationFunctionType.Exp,
                                                 scale=0.125)
                            for msk in (pd1[:, 0, 0:128], pd1[:, 1, 0:128],
                                        pd1[:, 1, 384:512], pd2[:, 0, 0:128]):
                                nc.gpsimd.tensor_tensor(out=msk, in0=msk, in1=mtri,
                                                        op=mybir.AluOpType.mult)
                            av(pd1[:, 0, :], s0, 0)
                            av(pd1[:, 1, 0:384], s0 + 1, 1)
                            av(pd2[:, 0, 0:256], s0 + 2, 2)
                            av(pd1[:, 1, 384:512], s0 + 3, 3)

                        # softmax denominator -> reciprocal -> normalize
                        r4 = rp.tile([128, 4, 1], F32, tag="r4", name="r4")
                        nc.vector.reciprocal(out=r4, in_=pav[:, :, 64:65])
                        ohn = ohp.tile([128, 4, 64], BF16, tag="ohn", name="ohn")
                        nc.vector.tensor_tensor(out=ohn, in0=pav[:, :, 0:64],
                                                in1=_bcast_last(r4, 64),
                                                op=mybir.AluOpType.mult)
                        if poht is None:
                            poht = pmm.tile([128, 4, 128], BF16, tag="mm",
                                            name="poht")
                        for tb in range(4):
                            nc.tensor.transpose(poht[hb:hb + 64, tb, :],
                                                ohn[:, tb, :], ident)
                        if 2 * g + half < len(splice_q):
                            proj_tile_q(splice_q[2 * g + half])
                    nc.vector.tensor_copy(
                        ohT_all[:, g, :].rearrange("p (a b) -> p a b", a=4), poht)

                # output projection for this t-chunk
                for jt in range(NTT):
                    ps_r = pmm.tile([128, 512], F32, tag="mm", name="psr")
                    for g in range(4):
                        nc.tensor.matmul(ps_r, wo_sb[:, g, jt * 128:(jt + 1) * 128],
                                         ohT_all[:, g, :], start=(g == 0),
                                         stop=(g == 3), skip_group_check=True)
                    rs = rsp.tile([128, 512], F32, tag="rs", name="rs")
                    if j <= 1 and jt % 2 == 1:
                        # tail chunks: ACT has slack there, DVE does not
                        nc.scalar.activation(
                            out=rs, in_=ps_r,
                            func=mybir.ActivationFunctionType.Copy)
                    else:
                        nc.vector.tensor_copy(rs, ps_r)
                    eng = nc.sync if jt % 2 == 0 else nc.gpsimd
                    eng.dma_start(
                        out=res[jt * 128:(jt + 1) * 128, j * 512:(j + 1) * 512],
                        in_=rs)

            # Reversed-chunk schedule: all kv tiles plus quarter-3 q are
            # projected first so the largest chunk (3) -- which carries 40%
            # of the exp work -- starts as early as possible and ACT is
            # front-loaded. The remaining q projections are spliced between
            # attention (g, half) units as PE filler during exp-bound
            # stretches, ordered so each later chunk finds its q ready.
            for c in range(NCH):
                load_quarter(c)
            # kv0/kv1 + chunk-3's q first so its scores (and ACT's exp
            # stream) start as early as possible; the remaining kv tiles
            # land just ahead of the score pairs that consume them.
            proj_tile_kv(0)
            proj_tile_kv(1)
            for ti in range(12, 16):
                proj_tile_q(ti)
            for st in range(2, NTT):
                proj_tile_kv(st)
            attention_chunk(3, [8, 9, 10, 11, 4, 5, 6, 7])
            attention_chunk(2, [0, 1, 2, 3])
            attention_chunk(1, [])
            attention_chunk(0, [])
    return nc


_NC_CACHE = {}


def _get_nc(causal: bool):
    if causal not in _NC_CACHE:
        _NC_CACHE[causal] = _build(causal)
    return _NC_CACHE[causal]


# ---------------------------------------------------------------------------
# Host wrapper
# ---------------------------------------------------------------------------

def kernel(x, cos, sin, mask, wq, wk, wv, wo):
    x = np.asarray(x, dtype=np.float32)
    cos = np.asarray(cos, dtype=np.float32)
    sin = np.asarray(sin, dtype=np.float32)
    mask = np.asarray(mask)
    wq = np.asarray(wq, dtype=np.float32)
    wk = np.asarray(wk, dtype=np.float32)
    wv = np.asarray(wv, dtype=np.float32)
    wo = np.asarray(wo, dtype=np.float32)

    m2 = mask[0, 0]
    tril = np.tril(np.ones((T, T), dtype=bool))
    if np.array_equal(m2, tril):
        causal = True
    elif m2.all():
        causal = False
    else:
        return _numpy_fallback(x, cos, sin, mask, wq, wk, wv, wo)

    _install_waitsplit()
    nc = _get_nc(causal)

    cexp = np.concatenate([cos, cos], axis=1).astype(np.float32)
    sexp = np.concatenate([-sin, sin], axis=1).astype(np.float32)

    in_maps = []
    for c in range(NCORES):
        b, jg = c // 4, c % 4
        heads = []
        for g in range(4):
            heads.append(8 * jg + g)
            heads.append(8 * jg + 4 + g)
        wq_rows = np.concatenate([wq[h * HD:(h + 1) * HD, :] for h in heads], axis=0)
        wo_cols = np.concatenate([wo[:, h * HD:(h + 1) * HD].T for h in heads], axis=0)
        kv = [2 * jg, 2 * jg + 1]
        wk_rows = np.concatenate([wk[k * HD:(k + 1) * HD, :] for k in kv], axis=0)
        wv_rows = np.concatenate([wv[k * HD:(k + 1) * HD, :] for k in kv], axis=0)
        wkv_cols = np.concatenate([wk_rows.T, wv_rows.T], axis=1)  # [D, 256]
        in_maps.append({
            "xt": np.ascontiguousarray(x[b].T).astype(NBF),
            "wq": np.ascontiguousarray(wq_rows.T).astype(NBF),
            "wkv": np.ascontiguousarray(wkv_cols).astype(NBF),
            "wo": np.ascontiguousarray(wo_cols).astype(NBF),
            "cexp": cexp,
            "sexp": sexp,
        })

    r = run_bass_kernel_spmd(nc, in_maps, core_ids=list(range(NCORES)))

    out = np.zeros((2, T, D), dtype=np.float32)
    for c in range(NCORES):
        out[c // 4] += r.results[c]["res"].T
    return out


def _numpy_fallback(x, cos, sin, mask, wq, wk, wv, wo):
    B = x.shape[0]
    NH, NKV = 32, 8
    q = (x @ wq.T).reshape(B, T, NH, HD).transpose(0, 2, 1, 3)
    k = (x @ wk.T).reshape(B, T, NKV, HD).transpose(0, 2, 1, 3)
    v = (x @ wv.T).reshape(B, T, NKV, HD).transpose(0, 2, 1, 3)

    def rope_np(t4):
        c = cos[None, None]
        s = sin[None, None]
        t1, t2 = t4[..., :32], t4[..., 32:]
        return np.concatenate([t1 * c - t2 * s, t2 * c + t1 * s], axis=-1)

    q, k = rope_np(q), rope_np(k)
    k = np.repeat(k, 4, axis=1)
    v = np.repeat(v, 4, axis=1)
    att = np.einsum("bhtd,bhsd->bhts", q, k) / np.sqrt(HD)
    att = np.where(mask, att, -np.inf)
    att = att - att.max(axis=-1, keepdims=True)
    p = np.exp(att)
    p /= p.sum(axis=-1, keepdims=True)
    o = np.einsum("bhts,bhsd->bhtd", p, v)
    o = o.transpose(0, 2, 1, 3).reshape(B, T, -1)
    return (o @ wo.T).astype(np.float32)



# revision 23
# speedup vs baseline: 1.0928x; 1.0928x over previous
"""GQA attention kernel for 8 Trainium2 cores (v3).

Problem: B=2, T=2048, D=2048, 32 q-heads, 8 kv-heads, head_dim=64, causal.

Sharding: core c = (b, jg) with b = c//4, jg = c%4. Each core handles batch b,
kv-heads {2jg, 2jg+1} and q-heads {8jg..8jg+7} (data parallel on B, tensor
parallel on heads; wq/wk/wv column-sharded, wo row-sharded). Each core returns
a partial output projection resT [D, T]; the host sums the 4 partials per
batch and transposes.

v3 design changes vs the 299us v2:
 - q/k/v projections run as fp8e4m3 DoubleRow matmuls with one-level
   residual correction: x = x8 + xd8, w = w8 + wd8 (host-quantized), and
   PSUM accumulates x8@w8 + x8@wd8 + xd8@w8 (the xd8@wd8 term is ~1e-5
   and dropped). 24 matmuls of 256-wide contraction at 0.5 cyc/col
   replace 16 bf16 matmuls at 1 cyc/col: 25% fewer PE cycles at bf16-level
   accuracy (verified 4.0e-3 vs 4.3e-3 all-bf16 in numpy).  Weights are
   host-scaled by 32 to clear the fp8 subnormal floor; the 1/32 is folded
   into the rope tables (q, k) and the vp copy (v) at zero device cost.
 - global software pipeline: attention for chunk 3 starts as soon as
   kv tiles 0-1 and q tiles 12-15 are projected (~6us), instead of after
   every projection (~45us).  kv tiles are demand-emitted inside the
   attention s-loop; all remaining projections and all output-projection
   tiles become "filler" work, popped between attention units whenever
   the emitted-PE-work clock falls behind the emitted-ACT-work clock, so
   the PE never idles while ACT grinds through the exp stream and ACT
   never starves behind a projection/outproj block.
 - input DMAs are batched 8-16x larger (DMA issue on SP/DVE costs
   ~600ns of sequencer time each) and ordered x[q0], x[q3], x[q1], x[q2]
   so the critical path (kv(0,1) -> q(12..15) -> attention(3)) unblocks
   at ~4us.
 - output-projection PSUM drains all moved off ACT (DVE/gpsimd) to keep
   ACT pure-exp; exp remains the only ACT work (~150us floor).
"""

import os
import sys

sys.path.insert(0, "/opt/trn_rl_repo")

import json

import numpy as np
import ml_dtypes

import concourse.bass as bass
import concourse.mybir as mybir
from concourse.tile import TileContext
from concourse.masks import make_identity
from concourse.bass_utils import run_bass_kernel_spmd

BF16 = mybir.dt.bfloat16
FP8 = mybir.dt.float8e4
F32 = mybir.dt.float32
DR = mybir.MatmulPerfMode.DoubleRow

T = 2048
D = 2048
HD = 64
NCORES = 8
KT = D // 128           # 16 contraction tiles
NTT = T // 128          # 16 time tiles
NCH = T // 512          # 4 chunks
NBF = ml_dtypes.bfloat16
E4 = ml_dtypes.float8_e4m3
WS = 32.0               # host-side weight scale before fp8 quantization

PE_NS = 1.0 / 2.4       # ns per PE cycle (ramped)

# debug bisect knobs
_NO_DEFER = bool(int(os.environ.get("K_NO_DEFER", "0")))
_NO_FILL = bool(int(os.environ.get("K_NO_FILL", "0")))
# deferring a unit-half's last AV group across the half boundary produced
# wrong results on hardware (sim-clean); flush at each half boundary.
_HALF_FLUSH = bool(int(os.environ.get("K_HALF_FLUSH", "1")))

# ---------------------------------------------------------------------------
# BIR post-pass: split multi-wait instructions into single-wait
# EventSemaphore carriers (the walrus build here allows one wait per inst).
# ---------------------------------------------------------------------------
_ws_ctr = [0]


def _split_waits_bytes(bir: bytes) -> bytes:
    d = json.loads(bir)
    for f in d.get("functions", []):
        for bb in f.get("blocks", []):
            out = []
            for inst in bb.get("instructions", []):
                si = inst.get("sync_info")
                waits = (si or {}).get("on_wait") or []
                if len(waits) > 1:
                    for w in waits[:-1]:
                        _ws_ctr[0] += 1
                        out.append({
                            "debug": inst.get("debug", 0),
                            "engine": inst["engine"],
                            "ins": [],
                            "name": f"WS-{_ws_ctr[0]}",
                            "opcode": "EventSemaphore",
                            "outs": [],
                            "sync_info": {"on_update": [], "on_wait": [w]},
                        })
                    si["on_wait"] = [waits[-1]]
                out.append(inst)
            bb["instructions"] = out
    return json.dumps(d).encode()


def _install_waitsplit():
    import concourse.bass2jax as b2j

    if getattr(b2j, "_waitsplit_installed", False):
        return
    orig = b2j._decompress_ant_bir
    b2j._decompress_ant_bir = lambda s: _split_waits_bytes(orig(s))
    b2j._waitsplit_installed = True


# ---------------------------------------------------------------------------
# Device program
# ---------------------------------------------------------------------------

def _bcast(ap2d, nh):
    """Insert a step-0 head dim into a [p, w] AP -> [p, nh, w]."""
    return bass.AP(tensor=ap2d.tensor, offset=ap2d.offset,
                   ap=[ap2d.ap[0], [0, nh], ap2d.ap[1]])


def _bcast_last(ap3d, w):
    """Append a step-0 last dim to a [p, n, 1] AP -> [p, n, w]."""
    return bass.AP(tensor=ap3d.tensor, offset=ap3d.offset,
                   ap=[ap3d.ap[0], ap3d.ap[1], [0, w]])


def _build(causal: bool):
    nc = bass.Bass()
    xt8 = nc.dram_tensor("xt8", [D, T], FP8, kind="ExternalInput")
    xtd8 = nc.dram_tensor("xtd8", [D, T], FP8, kind="ExternalInput")
    wq8 = nc.dram_tensor("wq8", [D, 512], FP8, kind="ExternalInput")
    wqd8 = nc.dram_tensor("wqd8", [D, 512], FP8, kind="ExternalInput")
    wkv8 = nc.dram_tensor("wkv8", [D, 256], FP8, kind="ExternalInput")
    wkvd8 = nc.dram_tensor("wkvd8", [D, 256], FP8, kind="ExternalInput")
    wo = nc.dram_tensor("wo", [512, D], BF16, kind="ExternalInput")
    cexp = nc.dram_tensor("cexp", [T, 64], F32, kind="ExternalInput")
    sexp = nc.dram_tensor("sexp", [T, 64], F32, kind="ExternalInput")
    res = nc.dram_tensor("res", [D, T], F32, kind="ExternalOutput")

    with TileContext(nc) as tc:
        with (
            tc.tile_pool(name="const", bufs=1) as const,
            tc.tile_pool(name="big", bufs=1) as big,
            tc.tile_pool(name="ropew", bufs=3) as ropew,
            tc.tile_pool(name="qtfp", bufs=3) as qtfp,
            tc.tile_pool(name="ptp", bufs=4) as ptp,
            tc.tile_pool(name="ohp", bufs=3) as ohp,
            tc.tile_pool(name="rp", bufs=3) as rp,
            tc.tile_pool(name="ohtp", bufs=4) as ohtp,
            tc.tile_pool(name="rsp", bufs=3) as rsp,
            tc.tile_pool(name="pmm", bufs=2, space="PSUM") as pmm,
            tc.tile_pool(name="psc", bufs=2, space="PSUM") as psc,
            tc.tile_pool(name="pav", bufs=2, space="PSUM") as pavp,
        ):
            # ---------------- constants ----------------
            ident = const.tile([128, 128], BF16)
            make_identity(nc, ident)

            mtri = None
            if causal:
                # keep col >= row (upper triangle incl diagonal) of a
                # [s_local, t_local] 128x128 tile
                mtri = const.tile([128, 128], BF16)
                nc.vector.memset(mtri, 1.0)
                nc.gpsimd.affine_select(
                    out=mtri, in_=mtri, pattern=[[1, 128]], base=0,
                    channel_multiplier=-1, compare_op=mybir.AluOpType.is_ge,
                    fill=0.0)

            # ---------------- SBUF-resident inputs -------------------------
            xt_sb = big.tile([128, 2, KT, T], FP8)      # [., {x8|xd8}, kt, t]
            wq_sb = big.tile([128, 2, KT, 512], FP8)
            wkv_sb = big.tile([128, 2, KT, 256], FP8)
            wo_sb = big.tile([128, 4, D], BF16)
            cexp_sb = const.tile([128, NTT, 64], F32)
            sexp_sb = const.tile([128, NTT, 64], F32)

            kT_sb = big.tile([128, NTT, 128], BF16)
            qT_all = big.tile([128, NTT, 512], BF16)
            # [kv0 | 1 | kv1 | 1]: v features for both kv heads plus the
            # all-ones denominator columns, one copy per s-tile
            vp = big.tile([128, NTT, 2, 65], BF16)
            nc.vector.memset(vp[:, :, :, 64:65], 1.0)

            # ---------------- input DMAs, batched + prioritized ------------
            # critical path: wkv + x[q0] -> kv(0,1); wq + x[q3] -> q(12..15)
            nc.sync.dma_start(
                out=wkv_sb[:, 0], in_=wkv8.rearrange("(n p) c -> p n c", p=128))
            nc.sync.dma_start(
                out=wkv_sb[:, 1], in_=wkvd8.rearrange("(n p) c -> p n c", p=128))
            # scalar (ACT) queue gets ONLY the early wq load: a DMA on the
            # ACT queue blocks the exp stream for its full transfer time.
            nc.scalar.dma_start(
                out=wq_sb[:, 0], in_=wq8.rearrange("(n p) c -> p n c", p=128))
            nc.scalar.dma_start(
                out=wq_sb[:, 1], in_=wqd8.rearrange("(n p) c -> p n c", p=128))
            nc.gpsimd.dma_start(
                out=cexp_sb, in_=cexp.rearrange("(n p) c -> p n c", p=128))
            nc.gpsimd.dma_start(
                out=sexp_sb, in_=sexp.rearrange("(n p) c -> p n c", p=128))

            xq = [nc.sync, nc.gpsimd]
            x8r = xt8.rearrange("(n p) c -> p n c", p=128)
            xd8r = xtd8.rearrange("(n p) c -> p n c", p=128)

            def load_quarter(c, qs):
                lo, hi = c * 512, (c + 1) * 512
                # 4 DMAs per quarter: {x8, xd8} x {kt 0-7, kt 8-15}
                for i, (v, kt0) in enumerate(((0, 0), (0, 8), (1, 0), (1, 8))):
                    src = x8r if v == 0 else xd8r
                    qs[i % len(qs)].dma_start(
                        out=xt_sb[:, v, kt0:kt0 + 8, lo:hi],
                        in_=src[:, kt0:kt0 + 8, lo:hi])

            # quarter 3 first: q(12..15) gate the first attention scores.
            # Neither touches the scalar queue (busy with the 6.4us wq load).
            load_quarter(3, [nc.sync, nc.gpsimd])
            load_quarter(0, [nc.gpsimd, nc.sync])

            def load_rest():
                load_quarter(1, xq)
                load_quarter(2, [nc.gpsimd, nc.sync])
                for g in range(4):
                    xq[g % 2].dma_start(
                        out=wo_sb[:, g, :], in_=wo[g * 128:(g + 1) * 128, :])

            # ---------------- emission bookkeeping --------------------------
            # Two-clock model of the emitted schedule: clk["pe"]/clk["act"]
            # estimate when each engine finishes everything emitted so far.
            # Attention emission keeps ACT saturated; whenever the PE clock
            # falls behind the ACT clock, filler work (projections, output
            # projection tiles) is popped to keep the PE busy.
            clk = {"pe": 0.0, "act": 0.0}
            SEMNS = 150.0

            def pe(cycles):
                clk["pe"] += cycles * PE_NS

            fillers = []        # list of thunks; each emits PE-heavy work
            # "mm"-ring discipline: while a unit's poht tile is live, at most
            # one other "mm"-tagged allocation may be emitted (2-slot ring).
            fill_budget = [None]   # None = unlimited; int = remaining

            def maybe_fill():
                if _NO_FILL:
                    return
                while fillers and clk["pe"] < clk["act"]:
                    if fill_budget[0] is not None and fill_budget[0] <= 0:
                        return
                    fillers.pop(0)()
                    if fill_budget[0] is not None:
                        fill_budget[0] -= 1

            # ---------------- rope -----------------------------------------
            def rope(ps3, out_bf, ti, nh):
                """RoPE in [t, f] layout. ps3: PSUM [128, nh, 64] f32 view ->
                out_bf: SBUF [128, nh*64] bf16. One DVE op to drain PSUM
                fast; the arithmetic runs on gpsimd from SBUF."""
                o3 = out_bf.rearrange("p (h k) -> p h k", h=nh)
                tmp = ropew.tile([128, nh, 64], F32, tag="ropeT", name="rt")
                nc.vector.tensor_copy(tmp, ps3)
                a = ropew.tile([128, nh, 64], F32, tag="ropeA", name="ra")
                bt = ropew.tile([128, nh, 64], F32, tag="ropeB", name="rb")
                cb = _bcast(cexp_sb[:, ti, :], nh)
                nc.gpsimd.tensor_tensor(out=a, in0=tmp, in1=cb, op=mybir.AluOpType.mult)
                sb_lo = _bcast(sexp_sb[:, ti, 0:32], nh)
                sb_hi = _bcast(sexp_sb[:, ti, 32:64], nh)
                nc.gpsimd.tensor_tensor(out=bt[:, :, 0:32], in0=tmp[:, :, 32:64],
                                        in1=sb_lo, op=mybir.AluOpType.mult)
                nc.gpsimd.tensor_tensor(out=bt[:, :, 32:64], in0=tmp[:, :, 0:32],
                                        in1=sb_hi, op=mybir.AluOpType.mult)
                nc.gpsimd.tensor_tensor(out=o3, in0=a, in1=bt, op=mybir.AluOpType.add)

            # ---------------- projections (3-term residual fp8 DR) ----------
            def proj_psum(ps, tcols, w_sb, wcols):
                first = True
                for xv, wv_ in ((0, 0), (0, 1), (1, 0)):
                    for kt in range(0, KT, 2):
                        last = (xv, wv_, kt) == (1, 0, KT - 2)
                        nc.tensor.matmul(
                            ps, xt_sb[:, xv, kt:kt + 2, tcols],
                            w_sb[:, wv_, kt:kt + 2, 0:wcols],
                            start=first, stop=last, perf_mode=DR)
                        first = False

            kv_done = [False] * NTT

            def proj_tile_kv(st):
                if kv_done[st]:
                    return
                kv_done[st] = True
                ps_kv = pmm.tile([128, 512], F32, tag="mm", name="pskv")
                proj_psum(ps_kv[:, 0:256], slice(st * 128, (st + 1) * 128),
                          wkv_sb, 256)
                pe(3 * 8 * 128)
                ktf = qtfp.tile([128, 128], BF16, tag="qtf", name="ktf")
                rope(ps_kv[:, 0:128].rearrange("p (h k) -> p h k", h=2),
                     ktf, st, 2)
                nc.sync.dma_start_transpose(kT_sb[:, st, :], ktf)
                # v with the 1/WS weight-scale fold
                nc.vector.tensor_scalar_mul(
                    vp[:, st, :, 0:64],
                    ps_kv[:, 128:256].rearrange("p (h k) -> p h k", h=2),
                    1.0 / WS)

            q_done = [False] * NTT

            def proj_tile_q(ti):
                if q_done[ti]:
                    return
                q_done[ti] = True
                ps_q = pmm.tile([128, 512], F32, tag="mm", name="psq")
                proj_psum(ps_q, slice(ti * 128, (ti + 1) * 128), wq_sb, 512)
                pe(3 * 8 * 256)
                qtf = qtfp.tile([128, 512], BF16, tag="qtf", name="qtf")
                rope(ps_q.rearrange("p (h k) -> p h k", h=8), qtf, ti, 8)
                c, tt = ti // 4, ti % 4
                for g in range(4):
                    nc.sync.dma_start_transpose(
                        qT_all[:, 4 * c + g, tt * 128:(tt + 1) * 128],
                        qtf[:, g * 128:(g + 1) * 128])

            # ---------------- output projection (one jt tile) ---------------
            def outproj_tile(ohT_all, j, jt):
                ps_r = pmm.tile([128, 512], F32, tag="mm", name="psr")
                for g in range(4):
                    nc.tensor.matmul(ps_r, wo_sb[:, g, jt * 128:(jt + 1) * 128],
                                     ohT_all[:, g, :], start=(g == 0),
                                     stop=(g == 3), skip_group_check=True)
                pe(4 * 512)
                rs = rsp.tile([128, 512], F32, tag="rs", name="rs")
                nc.vector.tensor_copy(rs, ps_r)
                eng = nc.sync if jt % 2 == 0 else nc.gpsimd
                eng.dma_start(
                    out=res[jt * 128:(jt + 1) * 128, j * 512:(j + 1) * 512],
                    in_=rs)

            # ------- attention, software-pipelined across score groups ------
            # Each "group" is (scores+exp emission) for a pair of s-tiles or
            # a diagonal trapezoid.  Its AV matmuls (which wait on the exp)
            # are deferred until after the NEXT group's scores+exp have been
            # emitted, so ACT rolls from one exp straight into the next and
            # the PE does the next group's scores while ACT works.
            pending = [None]

            def flush_pending():
                if pending[0] is not None:
                    fn = pending[0]
                    pending[0] = None
                    fn()

            def attention_chunk(j):
                # force the chunk's q projections to be emitted before any
                # score matmul that reads them (usually already popped as
                # filler; this is the correctness backstop).  Flush first so
                # the previous unit's poht is released ("mm" ring).
                if not all(q_done[4 * j:4 * j + 4]):
                    flush_pending()
                    fill_budget[0] = None
                    for ti in range(4 * j, 4 * j + 4):
                        proj_tile_q(ti)
                ohT_all = ohtp.tile([128, 4, 512], BF16, tag="oht", name="oht")
                for g in range(4):
                    qT = qT_all[:, 4 * j + g, :]
                    poht_box = [None]
                    for half in (0, 1):
                        hb = 64 * half
                        pav = pavp.tile([128, 4, 128], F32, tag="av", name="pav")
                        nc.vector.memset(pav[:, :, 0:65], 0.0)

                        def av(ptile, si, tb0, pav=pav, half=half):
                            for tb in range(tb0, 4):
                                nc.tensor.matmul(
                                    pav[:, tb, 0:65],
                                    ptile[:, (tb - tb0) * 128:(tb - tb0 + 1) * 128],
                                    vp[:, si, half, :], start=False, stop=False,
                                    skip_group_check=True)
                            pe((4 - tb0) * 65)

                        def finalize(pav=pav, hb=hb, g=g, half=half):
                            r4 = rp.tile([128, 4, 1], F32, tag="r4", name="r4")
                            nc.vector.reciprocal(out=r4, in_=pav[:, :, 64:65])
                            ohn = ohp.tile([128, 4, 64], BF16, tag="ohn",
                                           name="ohn")
                            nc.vector.tensor_tensor(out=ohn, in0=pav[:, :, 0:64],
                                                    in1=_bcast_last(r4, 64),
                                                    op=mybir.AluOpType.mult)
                            if poht_box[0] is None:
                                poht_box[0] = pmm.tile([128, 4, 128], BF16,
                                                       tag="mm", name="poht")
                            poht = poht_box[0]
                            for tb in range(4):
                                nc.tensor.transpose(poht[hb:hb + 64, tb, :],
                                                    ohn[:, tb, :], ident)
                            pe(4 * 128)
                            if half == 0:
                                # poht stays live: one "mm" ring slot left
                                fill_budget[0] = 1
                            else:
                                nc.vector.tensor_copy(
                                    ohT_all[:, g, :].rearrange(
                                        "p (a b) -> p a b", a=4), poht)
                                fill_budget[0] = None

                        ns_off = 4 * j if causal else NTT
                        groups = [("pair", sp) for sp in range(0, ns_off, 2)]
                        if causal:
                            groups.append(("diag", 4 * j))

                        for gi, (kind, s0) in enumerate(groups):
                            last = gi == len(groups) - 1
                            if kind == "pair":
                                proj_tile_kv(s0)
                                proj_tile_kv(s0 + 1)
                                ps2 = psc.tile([128, 2, 512], F32, tag="sc",
                                               name="pss")
                                for u in (0, 1):
                                    nc.tensor.matmul(
                                        ps2[:, u, :], kT_sb[hb:hb + 64, s0 + u, :],
                                        qT[hb:hb + 64, :],
                                        start=True, stop=True,
                                        skip_group_check=True)
                                pe(2 * 512)
                                pt2 = ptp.tile([128, 2, 512], BF16, tag="pt",
                                               name="pt")
                                nc.scalar.activation(
                                    out=pt2, in_=ps2,
                                    func=mybir.ActivationFunctionType.Exp,
                                    scale=0.125)
                                clk["act"] = max(clk["act"],
                                                 clk["pe"] + SEMNS) + 996
                                exp_done = clk["act"]

                                def avs(pt2=pt2, s0=s0, exp_done=exp_done,
                                        av=av, fin=finalize if last else None):
                                    clk["pe"] = max(clk["pe"], exp_done + SEMNS)
                                    av(pt2[:, 0, :], s0, 0)
                                    av(pt2[:, 1, :], s0 + 1, 0)
                                    if fin is not None:
                                        fin()
                            else:
                                for st in range(s0, s0 + 4):
                                    proj_tile_kv(st)
                                psd1 = psc.tile([128, 2, 512], F32, tag="sc",
                                                name="psd1")
                                psd2 = psc.tile([128, 2, 512], F32, tag="sc",
                                                name="psd2")
                                nc.tensor.matmul(psd1[:, 0, :],
                                                 kT_sb[hb:hb + 64, s0, :],
                                                 qT[hb:hb + 64, :],
                                                 start=True, stop=True,
                                                 skip_group_check=True)
                                nc.tensor.matmul(psd1[:, 1, 0:384],
                                                 kT_sb[hb:hb + 64, s0 + 1, :],
                                                 qT[hb:hb + 64, 128:512],
                                                 start=True, stop=True,
                                                 skip_group_check=True)
                                nc.tensor.matmul(psd1[:, 1, 384:512],
                                                 kT_sb[hb:hb + 64, s0 + 3, :],
                                                 qT[hb:hb + 64, 384:512],
                                                 start=True, stop=True,
                                                 skip_group_check=True)
                                nc.tensor.matmul(psd2[:, 0, 0:256],
                                                 kT_sb[hb:hb + 64, s0 + 2, :],
                                                 qT[hb:hb + 64, 256:512],
                                                 start=True, stop=True,
                                                 skip_group_check=True)
                                pe(512 + 384 + 128 + 256)
                                pd1 = ptp.tile([128, 2, 512], BF16, tag="pt",
                                               name="pd1")
                                nc.scalar.activation(
                                    out=pd1, in_=psd1,
                                    func=mybir.ActivationFunctionType.Exp,
                                    scale=0.125)
                                pd2 = ptp.tile([128, 2, 512], BF16, tag="pt",
                                               name="pd2")
                                nc.scalar.activation(
                                    out=pd2[:, 0, 0:256], in_=psd2[:, 0, 0:256],
                                    func=mybir.ActivationFunctionType.Exp,
                                    scale=0.125)
                                clk["act"] = max(clk["act"],
                                                 clk["pe"] + SEMNS) + 996 + 356
                                exp_done = clk["act"]

                                def avs(pd1=pd1, pd2=pd2, s0=s0,
                                        exp_done=exp_done, av=av,
                                        fin=finalize if last else None):
                                    clk["pe"] = max(clk["pe"], exp_done + SEMNS)
                                    for msk in (pd1[:, 0, 0:128],
                                                pd1[:, 1, 0:128],
                                                pd1[:, 1, 384:512],
                                                pd2[:, 0, 0:128]):
                                        nc.gpsimd.tensor_tensor(
                                            out=msk, in0=msk, in1=mtri,
                                            op=mybir.AluOpType.mult)
                                    av(pd1[:, 0, :], s0, 0)
                                    av(pd1[:, 1, 0:384], s0 + 1, 1)
                                    av(pd2[:, 0, 0:256], s0 + 2, 2)
                                    av(pd1[:, 1, 384:512], s0 + 3, 3)
                                    if fin is not None:
                                        fin()

                            flush_pending()
                            maybe_fill()
                            pending[0] = avs
                            if _NO_DEFER or (_HALF_FLUSH and last):
                                flush_pending()
                return ohT_all

            # ---------------- top-level schedule ----------------------------
            for ti in range(12, 16):
                proj_tile_q(ti)
            proj_tile_kv(0)
            proj_tile_kv(1)
            load_rest()

            # remaining projections become filler (kv tiles are also
            # demand-emitted by the attention s-loop as needed)
            for st in range(2, NTT):
                fillers.append(lambda st=st: proj_tile_kv(st))
            for ti in list(range(8, 12)) + list(range(4, 8)) + list(range(0, 4)):
                fillers.append(lambda ti=ti: proj_tile_q(ti))

            for j in (3, 2, 1, 0):
                ohT = attention_chunk(j)
                for jt in range(NTT):
                    fillers.append(
                        lambda ohT=ohT, j=j, jt=jt: outproj_tile(ohT, j, jt))

            # drain the pipeline and whatever filler work is left
            flush_pending()
            while fillers:
                fillers.pop(0)()
    return nc


_NC_CACHE = {}


def _get_nc(causal: bool):
    if causal not in _NC_CACHE:
        _NC_CACHE[causal] = _build(causal)
    return _NC_CACHE[causal]


# ---------------------------------------------------------------------------
# Host wrapper
# ---------------------------------------------------------------------------

def _q8_pair(a32):
    """fp8 quantize + fp8 residual of a float32 array."""
    a8 = a32.astype(E4)
    ad8 = (a32 - a8.astype(np.float32)).astype(E4)
    return a8, ad8


def kernel(x, cos, sin, mask, wq, wk, wv, wo):
    x = np.asarray(x, dtype=np.float32)
    cos = np.asarray(cos, dtype=np.float32)
    sin = np.asarray(sin, dtype=np.float32)
    mask = np.asarray(mask)
    wq = np.asarray(wq, dtype=np.float32)
    wk = np.asarray(wk, dtype=np.float32)
    wv = np.asarray(wv, dtype=np.float32)
    wo = np.asarray(wo, dtype=np.float32)

    m2 = mask[0, 0]
    tril = np.tril(np.ones((T, T), dtype=bool))
    if np.array_equal(m2, tril):
        causal = True
    elif m2.all():
        causal = False
    else:
        return _numpy_fallback(x, cos, sin, mask, wq, wk, wv, wo)

    _install_waitsplit()
    nc = _get_nc(causal)

    # rope tables fold the 1/WS weight scale
    cexp = (np.concatenate([cos, cos], axis=1) / WS).astype(np.float32)
    sexp = (np.concatenate([-sin, sin], axis=1) / WS).astype(np.float32)

    in_maps = []
    for c in range(NCORES):
        b, jg = c // 4, c % 4
        heads = []
        for g in range(4):
            heads.append(8 * jg + g)
            heads.append(8 * jg + 4 + g)
        wq_rows = np.concatenate([wq[h * HD:(h + 1) * HD, :] for h in heads], axis=0)
        wo_cols = np.concatenate([wo[:, h * HD:(h + 1) * HD].T for h in heads], axis=0)
        kv = [2 * jg, 2 * jg + 1]
        wk_rows = np.concatenate([wk[k * HD:(k + 1) * HD, :] for k in kv], axis=0)
        wv_rows = np.concatenate([wv[k * HD:(k + 1) * HD, :] for k in kv], axis=0)
        wkv_cols = np.concatenate([wk_rows.T, wv_rows.T], axis=1)  # [D, 256]

        xt8, xtd8 = _q8_pair(np.ascontiguousarray(x[b].T))
        wq8, wqd8 = _q8_pair(np.ascontiguousarray(wq_rows.T) * WS)
        wkv8, wkvd8 = _q8_pair(wkv_cols * WS)
        in_maps.append({
            "xt8": xt8, "xtd8": xtd8,
            "wq8": wq8, "wqd8": wqd8,
            "wkv8": wkv8, "wkvd8": wkvd8,
            "wo": np.ascontiguousarray(wo_cols).astype(NBF),
            "cexp": cexp,
            "sexp": sexp,
        })

    r = run_bass_kernel_spmd(nc, in_maps, core_ids=list(range(NCORES)))

    out = np.zeros((2, T, D), dtype=np.float32)
    for c in range(NCORES):
        out[c // 4] += r.results[c]["res"].T
    return out


def _numpy_fallback(x, cos, sin, mask, wq, wk, wv, wo):
    B = x.shape[0]
    NH, NKV = 32, 8
    q = (x @ wq.T).reshape(B, T, NH, HD).transpose(0, 2, 1, 3)
    k = (x @ wk.T).reshape(B, T, NKV, HD).transpose(0, 2, 1, 3)
    v = (x @ wv.T).reshape(B, T, NKV, HD).transpose(0, 2, 1, 3)

    def rope_np(t4):
        c = cos[None, None]
        s = sin[None, None]
        t1, t2 = t4[..., :32], t4[..., 32:]
        return np.concatenate([t1 * c - t2 * s, t2 * c + t1 * s], axis=-1)

    q, k = rope_np(q), rope_np(k)
    k = np.repeat(k, 4, axis=1)
    v = np.repeat(v, 4, axis=1)
    att = np.einsum("bhtd,bhsd->bhts", q, k) / np.sqrt(HD)
    att = np.where(mask, att, -np.inf)
    att = att - att.max(axis=-1, keepdims=True)
    p = np.exp(att)
    p /= p.sum(axis=-1, keepdims=True)
    o = np.einsum("bhts,bhsd->bhtd", p, v)
    o = o.transpose(0, 2, 1, 3).reshape(B, T, -1)
    return (o @ wo.T).astype(np.float32)
